# revision 3
# baseline (speedup 1.0000x reference)
"""CrossEncoder4FutureTrajectory Trainium2 kernel.

Strategy:
  - Data-parallel over batch B=16 across 8 NeuronCores (2 batches/core).
  - All activations kept TRANSPOSED in SBUF ([hidden, tokens]) so the chained
    projections (stationary = weight block) never need transposes.
  - Scores computed directly transposed ([key, query]) so the attention
    matmul (contraction over keys) needs no transpose either; softmax runs
    without max-subtraction (scores are in [-20, 20] for this model), the
    denominator and layernorm stats are partition-dim sums done on the PE
    with a ones-vector stationary operand.
  - Masks are applied multiplicatively post-exp (u8 0/1), which matches the
    reference's where(mask, -1e12)+softmax+zero exactly (no all-masked rows).
  - All matmuls in float32r (full PE rate at moving-dim >= 256, ~tf32
    precision); the 1/sqrt(H) score scale is folded into the last k-proj
    weight stage on the host.
"""

import sys
from contextlib import ExitStack

import numpy as np

for _p in ("/opt/trn_rl_repo", "/root/.axon_site/_ro/trn_rl_repo"):
    if _p not in sys.path:
        sys.path.append(_p)

import concourse.bass as bass
import concourse.tile as tile
from concourse import bacc, mybir
from concourse import bass_utils
from concourse.masks import make_identity

F32 = mybir.dt.float32
F32R = mybir.dt.float32r
U8 = mybir.dt.uint8
AF = mybir.ActivationFunctionType
ALU = mybir.AluOpType

B, NF, NH, NG, H, L = 16, 1024, 1024, 2048, 256, 4
NCORES = 8
BPC = B // NCORES  # batches per core
EPS = 1e-5


def _proj_chain_T(nc, ctx, pools, x2, w_sb, b_sb, l, n_tok, dest):
    """3-stage chained projection fully in transposed layout.

    x2:    SBUF [128, 2, n_tok] f32r  (input, transposed)
    w_sb:  SBUF [128, 3, 2, 256] f32r (stationary weight blocks, [p, stage, kc, m])
    b_sb:  SBUF [128, L, 3, 2] f32    (per-partition biases)
    dest:  SBUF [128, 2, n_tok] f32r  (output, transposed)
    """
    psA, tmp_pool = pools["psA"], pools["projt"]
    CH = 1024  # token chunk
    n_ch = n_tok // CH
    for c in range(n_ch):
        src = x2
        for s in range(3):
            out_t = dest if s == 2 else tmp_pool.tile([128, 2, CH], F32R, tag="projt",
                                                      name=f"pt{s}")
            for mc in range(2):
                for it in range(CH // 512):
                    ps = psA.tile([128, 512], F32, tag="ps", name="ps_proj")
                    for kc in range(2):
                        nc.tensor.matmul(
                            ps,
                            w_sb[:, s, kc, mc * 128:(mc + 1) * 128],
                            src[:, kc, c * CH + it * 512: c * CH + (it + 1) * 512]
                            if s == 0 else src[:, kc, it * 512:(it + 1) * 512],
                            start=(kc == 0), stop=(kc == 1),
                        )
                    dst_sl = (out_t[:, mc, c * CH + it * 512: c * CH + (it + 1) * 512]
                              if s == 2 else out_t[:, mc, it * 512:(it + 1) * 512])
                    nc.scalar.activation(dst_sl, ps, AF.Identity,
                                         bias=b_sb[:, l, s, mc:mc + 1], scale=1.0)
            src = out_t


def _v_chain_natural(nc, ctx, pools, x2, w_sb, b_sb, l, bv_row, n_tok, v_sb):
    """v projection: stages 0/1 transposed-chain, stage 2 emitted in natural
    [key, hidden] layout (stationary = activation block, moving = weights)."""
    psA, tmp_pool, bc_pool = pools["psA"], pools["projt"], pools["bc"]
    CH = 1024
    n_ch = n_tok // CH
    # broadcast bias row for stage 2 (free-dim bias in natural layout)
    bvB = bc_pool.tile([128, 256], F32, tag="bvB", name="bvB", bufs=2)
    nc.gpsimd.partition_broadcast(bvB, bv_row)
    for c in range(n_ch):
        src = x2
        for s in range(2):
            out_t = tmp_pool.tile([128, 2, CH], F32R, tag="projt", name=f"vt{s}")
            for mc in range(2):
                for it in range(CH // 512):
                    ps = psA.tile([128, 512], F32, tag="ps", name="ps_vproj")
                    for kc in range(2):
                        nc.tensor.matmul(
                            ps,
                            w_sb[:, s, kc, mc * 128:(mc + 1) * 128],
                            src[:, kc, c * CH + it * 512: c * CH + (it + 1) * 512]
                            if s == 0 else src[:, kc, it * 512:(it + 1) * 512],
                            start=(kc == 0), stop=(kc == 1),
                        )
                    nc.scalar.activation(out_t[:, mc, it * 512:(it + 1) * 512], ps,
                                         AF.Identity, bias=b_sb[:, l, s, mc:mc + 1],
                                         scale=1.0)
            src = out_t
        # stage 2: natural layout out [j, d]; lhsT = src block [d'=128, j=128]
        for jb in range(CH // 128):
            ps = psA.tile([128, 256], F32, tag="ps", name="ps_vnat")
            for kc in range(2):
                nc.tensor.matmul(
                    ps,
                    src[:, kc, jb * 128:(jb + 1) * 128],
                    w_sb[:, 2, kc, :],
                    start=(kc == 0), stop=(kc == 1),
                )
            nc.vector.tensor_add(v_sb[:, c * (CH // 128) + jb, :], ps, bvB)


def build_program():
    nc = bacc.Bacc("TRN2", target_bir_lowering=False, debug=False,
                   enable_asserts=False)

    futT = nc.dram_tensor("futT", [BPC, H, NF], F32R, kind="ExternalInput").ap()
    hisT = nc.dram_tensor("hisT", [BPC, H, NH], F32R, kind="ExternalInput").ap()
    graT = nc.dram_tensor("graT", [BPC, H, NG], F32R, kind="ExternalInput").ap()
    mhfT = nc.dram_tensor("mhfT", [BPC, NH, NF], U8, kind="ExternalInput").ap()
    mfgT = nc.dram_tensor("mfgT", [BPC, NG, NF], U8, kind="ExternalInput").ap()
    Wq = nc.dram_tensor("Wq", [L, 3, H, H], F32R, kind="ExternalInput").ap()
    Wk = nc.dram_tensor("Wk", [L, 3, H, H], F32R, kind="ExternalInput").ap()
    Wv = nc.dram_tensor("Wv", [L, 3, H, H], F32R, kind="ExternalInput").ap()
    bq = nc.dram_tensor("bq", [L, 3, H], F32, kind="ExternalInput").ap()
    bk = nc.dram_tensor("bk", [L, 3, H], F32, kind="ExternalInput").ap()
    bv = nc.dram_tensor("bv", [L, 3, H], F32, kind="ExternalInput").ap()
    gamma = nc.dram_tensor("gamma", [L, H], F32, kind="ExternalInput").ap()
    beta = nc.dram_tensor("beta", [L, H], F32, kind="ExternalInput").ap()
    out = nc.dram_tensor("out", [BPC, NF, H], F32, kind="ExternalOutput").ap()

    with tile.TileContext(nc) as tc:
        with ExitStack() as ctx:
            # ---- pools
            singles = ctx.enter_context(tc.tile_pool(name="singles", bufs=1))
            wpools = ctx.enter_context(tc.tile_pool(name="wpools", bufs=1))
            xpool = ctx.enter_context(tc.tile_pool(name="xpool", bufs=2))
            x2pool = ctx.enter_context(tc.tile_pool(name="x2pool", bufs=1))
            qkv = ctx.enter_context(tc.tile_pool(name="qkv", bufs=1))
            projt = ctx.enter_context(tc.tile_pool(name="projt", bufs=2))
            wsoft = ctx.enter_context(tc.tile_pool(name="wsoft", bufs=2))
            mpool = ctx.enter_context(tc.tile_pool(name="mpool", bufs=4))
            opool = ctx.enter_context(tc.tile_pool(name="opool", bufs=4))
            tmp = ctx.enter_context(tc.tile_pool(name="tmp", bufs=4))
            bc = ctx.enter_context(tc.tile_pool(name="bc", bufs=6))
            rows = ctx.enter_context(tc.tile_pool(name="rows", bufs=2))
            outp = ctx.enter_context(tc.tile_pool(name="outp", bufs=2))
            psA = ctx.enter_context(tc.tile_pool(name="psA", bufs=2, space="PSUM"))
            psDen = ctx.enter_context(tc.tile_pool(name="psDen", bufs=2, space="PSUM"))
            psO = ctx.enter_context(tc.tile_pool(name="psO", bufs=4, space="PSUM"))
            pools = {"psA": psA, "projt": projt, "bc": bc}

            # ---- one-time constants
            ones_f = singles.tile([128, 1], F32, tag="ones_f")
            nc.vector.memset(ones_f, 1.0)
            ones = singles.tile([128, 1], F32R, tag="ones")
            nc.vector.tensor_copy(ones, ones_f)
            eps_t = singles.tile([1, 1], F32, tag="eps")
            nc.vector.memset(eps_t, EPS)
            ident = singles.tile([128, 128], F32, tag="ident")
            make_identity(nc, ident)

            # ---- biases / ln params, all layers at once (small)
            bq_sb = singles.tile([128, L, 3, 2], F32, tag="bq")
            nc.sync.dma_start(bq_sb, bq.rearrange("l s (c p) -> p l s c", p=128))
            bk_sb = singles.tile([128, L, 3, 2], F32, tag="bk")
            nc.sync.dma_start(bk_sb, bk.rearrange("l s (c p) -> p l s c", p=128))
            bv_sb = singles.tile([128, L, 3, 2], F32, tag="bv")
            nc.sync.dma_start(bv_sb, bv.rearrange("l s (c p) -> p l s c", p=128))
            gam_sb = singles.tile([128, L, 2], F32, tag="gam")
            nc.sync.dma_start(gam_sb, gamma.rearrange("l (c p) -> p l c", p=128))
            bet_sb = singles.tile([128, L, 2], F32, tag="bet")
            nc.sync.dma_start(bet_sb, beta.rearrange("l (c p) -> p l c", p=128))
            bv_rows = singles.tile([1, L, 256], F32, tag="bvrow")
            nc.sync.dma_start(bv_rows, bv[:, 2:3, :].rearrange("l o d -> o l d"))

            for b in range(BPC):
                # ---- load transposed activations for this batch
                fut_sb = x2pool.tile([128, 2, NF], F32R, tag="fut", name="fut_sb")
                for cc in range(2):
                    nc.sync.dma_start(fut_sb[:, cc, :],
                                      futT[b, cc * 128:(cc + 1) * 128, :])
                his_sb = x2pool.tile([128, 2, NH], F32R, tag="his", name="his_sb")
                for cc in range(2):
                    nc.sync.dma_start(his_sb[:, cc, :],
                                      hisT[b, cc * 128:(cc + 1) * 128, :])
                gra_sb = x2pool.tile([128, 2, NG], F32R, tag="gra", name="gra_sb")
                for cc in range(2):
                    nc.sync.dma_start(gra_sb[:, cc, :],
                                      graT[b, cc * 128:(cc + 1) * 128, :])

                x1 = fut_sb
                for l in range(L):
                    x2, n_k, mT = ((his_sb, NH, mhfT) if l < 2
                                   else (gra_sb, NG, mfgT))
                    NJ = n_k // 128

                    # ---- per-layer weights
                    wq_sb = wpools.tile([128, 3, 2, 256], F32R, tag="wq", name="wq_sb")
                    nc.sync.dma_start(
                        wq_sb, Wq[l].rearrange("s (c p) m -> p s c m", p=128))
                    wk_sb = wpools.tile([128, 3, 2, 256], F32R, tag="wk", name="wk_sb")
                    nc.sync.dma_start(
                        wk_sb, Wk[l].rearrange("s (c p) m -> p s c m", p=128))
                    wv_sb = wpools.tile([128, 3, 2, 256], F32R, tag="wv", name="wv_sb")
                    nc.sync.dma_start(
                        wv_sb, Wv[l].rearrange("s (c p) m -> p s c m", p=128))

                    # ---- projections
                    kT = qkv.tile([128, 2, n_k], F32R, tag="kT", name="kT")
                    _proj_chain_T(nc, ctx, pools, x2, wk_sb, bk_sb, l, n_k, kT)
                    v_sb = qkv.tile([128, NJ, 256], F32R, tag="v_sb", name="v_sb")
                    _v_chain_natural(nc, ctx, pools, x2, wv_sb, bv_sb, l,
                                     bv_rows[:, l, :], n_k, v_sb)
                    qT = qkv.tile([128, 2, NF], F32R, tag="qT", name="qT")
                    _proj_chain_T(nc, ctx, pools, x1, wq_sb, bq_sb, l, NF, qT)

                    # ---- attention j-loop (keys on partitions)
                    dps = [psDen.tile([1, 512], F32, tag="den", name=f"dps{it}")
                           for it in range(2)]
                    ops = [[psO.tile([128, 512], F32, tag="po", name=f"ops{dc}{it}")
                            for it in range(2)] for dc in range(2)]
                    for jc in range(NJ):
                        m_t = mpool.tile([128, NF], U8, tag="m", name="m_t")
                        nc.sync.dma_start(m_t, mT[b, jc * 128:(jc + 1) * 128, :])
                        w_t = wsoft.tile([128, NF], F32R, tag="w", name="w_t")
                        for it in range(2):
                            ps_s = psA.tile([128, 512], F32, tag="ps", name="ps_s")
                            for dc in range(2):
                                nc.tensor.matmul(
                                    ps_s,
                                    kT[:, dc, jc * 128:(jc + 1) * 128],
                                    qT[:, dc, it * 512:(it + 1) * 512],
                                    start=(dc == 0), stop=(dc == 1),
                                )
                            nc.scalar.activation(w_t[:, it * 512:(it + 1) * 512],
                                                 ps_s, AF.Exp)
                        nc.vector.tensor_tensor(w_t, w_t, m_t, ALU.mult)
                        first, last = (jc == 0), (jc == NJ - 1)
                        for it in range(2):
                            nc.tensor.matmul(dps[it], ones,
                                             w_t[:, it * 512:(it + 1) * 512],
                                             start=first, stop=last,
                                             skip_group_check=True)
                        for dc in range(2):
                            for it in range(2):
                                nc.tensor.matmul(
                                    ops[dc][it],
                                    v_sb[:, jc, dc * 128:(dc + 1) * 128],
                                    w_t[:, it * 512:(it + 1) * 512],
                                    start=first, stop=last,
                                    skip_group_check=True)

                    # ---- normalize + layernorm + residual
                    o_sb = opool.tile([128, 2, NF], F32R, tag="o", name="o_sb",
                                      bufs=1)
                    xn = xpool.tile([128, 2, NF], F32R, tag="xT", name="xn")
                    for it in range(2):
                        sl = slice(it * 512, (it + 1) * 512)
                        rec = rows.tile([1, 512], F32, tag="rec", name="rec")
                        nc.vector.reciprocal(rec, dps[it])
                        recB = bc.tile([128, 512], F32, tag="bc", name="recB")
                        nc.gpsimd.partition_broadcast(recB, rec)
                        for dc in range(2):
                            nc.vector.tensor_tensor(o_sb[:, dc, sl], ops[dc][it],
                                                    recB, ALU.mult)
                        # stats: sums over hidden (partitions) via ones-matmul
                        mu_ps = psDen.tile([1, 512], F32, tag="den", name="mu_ps")
                        for dc in range(2):
                            nc.tensor.matmul(mu_ps, ones, o_sb[:, dc, sl],
                                             start=(dc == 0), stop=(dc == 1))
                        ss_ps = psA.tile([128, 512], F32, tag="ps", name="ss_ps")
                        ss_r = ss_ps[0:1, :]
                        for dc in range(2):
                            sq = tmp.tile([128, 512], F32R, tag="tmp", name="sq")
                            nc.scalar.square(sq, o_sb[:, dc, sl])
                            nc.tensor.matmul(ss_r, ones, sq,
                                             start=(dc == 0), stop=(dc == 1))
                        # finalize stats (mu to SBUF; var/sd in-place in PSUM row)
                        mu_r = rows.tile([1, 512], F32, tag="mur", name="mu_r")
                        nc.vector.tensor_single_scalar(mu_r, mu_ps, 1.0 / H,
                                                       ALU.mult)
                        nc.vector.tensor_single_scalar(ss_r, ss_r, 1.0 / H, ALU.mult)
                        tmp_r = rows.tile([1, 512], F32, tag="tmpr", name="tmp_r")
                        nc.vector.scalar_tensor_tensor(tmp_r, mu_r, -1.0, mu_r,
                                                       ALU.mult, ALU.mult)
                        nc.vector.tensor_tensor(ss_r, ss_r, tmp_r, ALU.add)
                        nc.scalar.activation(ss_r, ss_r, AF.Sqrt, bias=eps_t,
                                             scale=1.0)
                        rstd = rows.tile([1, 512], F32, tag="rstd", name="rstd")
                        nc.vector.reciprocal(rstd, ss_r)
                        nmr = rows.tile([1, 512], F32, tag="nmr", name="nmr")
                        nc.vector.scalar_tensor_tensor(nmr, mu_r, -1.0, rstd,
                                                       ALU.mult, ALU.mult)
                        rstdB = bc.tile([128, 512], F32, tag="bc", name="rstdB")
                        nc.gpsimd.partition_broadcast(rstdB, rstd)
                        nmrB = bc.tile([128, 512], F32, tag="bc", name="nmrB")
                        nc.gpsimd.partition_broadcast(nmrB, nmr)
                        for dc in range(2):
                            t1 = tmp.tile([128, 512], F32, tag="tmp", name="t1")
                            nc.vector.scalar_tensor_tensor(
                                t1, o_sb[:, dc, sl], gam_sb[:, l, dc:dc + 1],
                                rstdB, ALU.mult, ALU.mult)
                            u = tmp.tile([128, 512], F32, tag="tmp", name="u")
                            nc.vector.tensor_scalar(
                                u, nmrB, gam_sb[:, l, dc:dc + 1],
                                bet_sb[:, l, dc:dc + 1], ALU.mult, ALU.add)
                            nc.vector.tensor_tensor(u, u, x1[:, dc, sl], ALU.add)
                            nc.vector.tensor_tensor(xn[:, dc, sl], t1, u, ALU.add)
                    x1 = xn

                # ---- final residual + transpose back + store
                y = opool.tile([128, 2, NF], F32, tag="y", name="y", bufs=1)
                for dc in range(2):
                    nc.vector.tensor_add(y[:, dc, :], x1[:, dc, :], fut_sb[:, dc, :])
                for ic in range(NF // 128):
                    o_nat = outp.tile([128, 256], F32, tag="onat", name="o_nat")
                    for dc in range(2):
                        tp = psA.tile([128, 128], F32, tag="ps", name="tp")
                        nc.tensor.transpose(tp, y[:, dc, ic * 128:(ic + 1) * 128],
                                            ident)
                        nc.scalar.copy(o_nat[:, dc * 128:(dc + 1) * 128], tp)
                    nc.sync.dma_start(out[b, ic * 128:(ic + 1) * 128, :], o_nat)

    nc.compile()
    return nc


_CACHED_NC = None


def _get_nc():
    global _CACHED_NC
    if _CACHED_NC is None:
        _CACHED_NC = build_program()
    return _CACHED_NC


def _prep_in_maps(inputs):
    f32 = np.float32
    future = np.asarray(inputs["future"], f32)
    history = np.asarray(inputs["history"], f32)
    graph = np.asarray(inputs["graph"], f32)
    mask_hf = np.asarray(inputs["mask_hf"])
    mask_fg = np.asarray(inputs["mask_fg"])
    Wq = np.asarray(inputs["Wq"], f32)
    Wk = np.asarray(inputs["Wk"], f32).copy()
    Wv = np.asarray(inputs["Wv"], f32)
    bq = np.asarray(inputs["bq"], f32)
    bk = np.asarray(inputs["bk"], f32).copy()
    bv = np.asarray(inputs["bv"], f32)
    gamma = np.asarray(inputs["gamma"], f32)
    beta = np.asarray(inputs["beta"], f32)

    # fold 1/sqrt(H) into the final k-projection stage
    s = 1.0 / np.sqrt(np.float32(H))
    Wk[:, 2] *= s
    bk[:, 2] *= s

    futT = np.ascontiguousarray(future.transpose(0, 2, 1))
    hisT = np.ascontiguousarray(history.transpose(0, 2, 1))
    graT = np.ascontiguousarray(graph.transpose(0, 2, 1))
    mhfT = np.ascontiguousarray((~mask_hf).transpose(0, 2, 1)).astype(np.uint8)
    mfgT = np.ascontiguousarray((~mask_fg).transpose(0, 2, 1)).astype(np.uint8)

    shared = {"Wq": Wq, "Wk": Wk, "Wv": Wv, "bq": bq, "bk": bk, "bv": bv,
              "gamma": gamma, "beta": beta}
    in_maps = []
    for c in range(NCORES):
        sl = slice(c * BPC, (c + 1) * BPC)
        in_maps.append({
            "futT": futT[sl], "hisT": hisT[sl], "graT": graT[sl],
            "mhfT": mhfT[sl], "mfgT": mfgT[sl], **shared,
        })
    return in_maps


def run(inputs, trace=False):
    nc = _get_nc()
    in_maps = _prep_in_maps(inputs)
    res = bass_utils.run_bass_kernel_spmd(
        nc, in_maps, core_ids=list(range(NCORES)), trace=trace)
    out = np.concatenate([res.results[c]["out"] for c in range(NCORES)], axis=0)
    return out.astype(np.float32), res


def kernel(**inputs):
    out, _ = run(inputs, trace=False)
    return out
